# revision 1
# baseline (speedup 1.0000x reference)
"""Trainium2 Bass kernel for nn_DressedQuantumNet (262144 x 64 -> 262144 x 1).

Math reduction (host, params only): the 4-qubit circuit after the per-sample
input RY layer is a FIXED 16x16 linear map U (depends only on q_params).
With the product state psi_w = (cos phi_w, sin phi_w),
phi_w = (pi/4)*(tanh(u_w)+1), u = x @ pre_w.T + pre_b:

    out = psi^T O psi + post_b,     O = sum_w post_w[w] * U^T Z_w U.

Device pipeline per core (32768 samples), fp16 data paths / fp32 psum:
  DMA-cast x->fp16 -> PE transpose -> pre-matmul (K=128: 2 sample-groups x
  64 feats, M=32 zero-padded) -> tanh(+pre_b) on ACT -> PE transpose
  (sample-major regroup) -> sin x2 on ACT -> pair/state products -> PE
  transpose -> block-diag O matmul -> elementwise dot -> reduce-matmul ->
  output transpose -> contiguous DMA out.

Sample bookkeeping: sample s = 8192*m + 64*p + k, k = 32*ut + 8*v + 2*cc + j.
Sample-major coords: s = 8192*(Q//64) + 64*mu + (Q%64) with mu=p,
Q = 64*m + k. Back half: Q = 32*w + 8*eb + qt.
"""
import sys

import numpy as np

for _p in ("/opt/trn_rl_repo",):
    if _p not in sys.path:
        sys.path.insert(0, _p)

import concourse.bass as bass
import concourse.bacc as bacc
import concourse.hw_specs as _hw_specs

_orig_get_act_tables = _hw_specs.get_activation_tables


def _pinned_act_tables(module_arch):
    tabs = _orig_get_act_tables(module_arch)
    if "silu_and_others" in tabs:
        tabs = {k: (v if k == "silu_and_others" else set())
                for k, v in tabs.items()}
    return tabs


bacc.get_activation_tables = _pinned_act_tables
import concourse.mybir as mybir
from concourse import tile
from concourse.bass_utils import run_bass_kernel_spmd

AF = mybir.ActivationFunctionType
ALU = mybir.AluOpType
F32 = mybir.dt.float32
F16 = mybir.dt.float16

N_CORES = 8
BATCH = 262144
S = BATCH // N_CORES          # 32768 samples per core
NM = S // 8192                # 4 macro-tiles per core
N_QUBITS = 4
Q_DEPTH = 6
IN_F = 64

TRACE = False                 # test.py sets True to collect a profile
LAST_RESULTS = None

# ---------------------------------------------------------------- host math


def _ry(theta):
    c, s = np.cos(theta / 2), np.sin(theta / 2)
    return np.array([[c, -s], [s, c]], dtype=np.float64)


def _lift1(gate, wire):
    ops = [np.eye(2)] * N_QUBITS
    ops[wire] = gate
    out = ops[0]
    for o in ops[1:]:
        out = np.kron(out, o)
    return out


def _cnot(ctrl, tgt):
    U = np.zeros((16, 16))
    for i in range(16):
        bits = [(i >> (N_QUBITS - 1 - w)) & 1 for w in range(N_QUBITS)]
        if bits[ctrl] == 1:
            bits[tgt] ^= 1
        j = sum(b << (N_QUBITS - 1 - w) for w, b in enumerate(bits))
        U[j, i] = 1.0
    return U


def quad_form(q_params, post_w):
    """O (16x16 fp64): out = psi^T O psi + post_b."""
    qw = np.asarray(q_params, dtype=np.float64).reshape(Q_DEPTH, N_QUBITS)
    U = np.eye(16)
    for k in range(Q_DEPTH):
        U = _cnot(0, 1) @ U
        U = _cnot(2, 3) @ U
        U = _cnot(1, 2) @ U
        for w in range(N_QUBITS):
            U = _lift1(_ry(qw[k, w]), w) @ U
    Z = np.diag([1.0, -1.0])
    O = np.zeros((16, 16))
    pw = np.asarray(post_w, dtype=np.float64).reshape(-1)
    for w in range(N_QUBITS):
        O += pw[w] * (U.T @ _lift1(Z, w) @ U)
    return O


def _consts(pre_w, pre_b, q_params, post_w):
    # Wstack (128, 32) f16: [64j + f, 4j + i] = pre_w[i, f]; rest zero.
    wstack = np.zeros((128, 32), dtype=np.float32)
    for j in range(2):
        for i in range(4):
            wstack[64 * j:64 * j + 64, 4 * j + i] = pre_w[i, :]
    # bias (128, 1) f32: row r -> pre_b[r % 4]
    biast = np.tile(np.asarray(pre_b, np.float32).reshape(4), 32)[:, None]
    biast = np.ascontiguousarray(biast, dtype=np.float32)
    # Mbd (128, 128) f16: blockdiag 8 x O (O symmetric)
    O = quad_form(q_params, post_w)
    mbd = np.zeros((128, 128), dtype=np.float32)
    for g in range(8):
        mbd[16 * g:16 * g + 16, 16 * g:16 * g + 16] = O
    # Rsum (128, 32) f16: [16q + st, q] = 1 for q in [0,8); rest zero.
    rsum = np.zeros((128, 32), dtype=np.float32)
    for q in range(8):
        rsum[16 * q:16 * q + 16, q] = 1.0
    ident16 = np.eye(128, dtype=np.float16)
    ident32 = np.eye(128, dtype=np.float32)
    return (wstack.astype(np.float16), biast, mbd.astype(np.float16),
            rsum.astype(np.float16), ident16, ident32)


# ---------------------------------------------------------------- program


def build(nm=NM, post_b=0.0):
    nc = bacc.Bacc()
    s_core = nm * 8192
    NQ = nm * 64                       # Q-column count (= s_core / 128)

    x = nc.declare_dram_parameter("x", (s_core, IN_F), F32, isOutput=False)
    y = nc.declare_dram_parameter("y", (s_core, 1), F32, isOutput=True)
    wstack_d = nc.declare_dram_parameter("wstack", (128, 32), F16, isOutput=False)
    bias_d = nc.declare_dram_parameter("biast", (128, 1), F32, isOutput=False)
    mbd_d = nc.declare_dram_parameter("mbd", (128, 128), F16, isOutput=False)
    rsum_d = nc.declare_dram_parameter("rsum", (128, 32), F16, isOutput=False)

    def xbar(out_ap, in_ap):
        nc.sync.dma_start(out_ap, in_ap, transpose=True)

    with tile.TileContext(nc) as tc:
        with (
            tc.tile_pool(name="const", bufs=1) as cpool,
            tc.tile_pool(name="xin", bufs=2) as xpool,
            tc.tile_pool(name="xt2p", bufs=3) as xtpool,
            tc.tile_pool(name="sb16", bufs=3) as spool,
            tc.tile_pool(name="pers", bufs=1) as ppool,
            tc.tile_pool(name="psu", bufs=3, space="PSUM") as ps_u,
            tc.tile_pool(name="psmy", bufs=3, space="PSUM") as ps_my,
        ):
            # constants
            wstack = cpool.tile([128, 32], F16, tag="wstack")
            biast = cpool.tile([128, 1], F32, tag="biast")
            mbd = cpool.tile([128, 128], F16, tag="mbd")
            rsum = cpool.tile([128, 32], F16, tag="rsum")
            nc.sync.dma_start(wstack[:], wstack_d[:])
            nc.sync.dma_start(biast[:], bias_d[:])
            nc.sync.dma_start(mbd[:], mbd_d[:])
            nc.sync.dma_start(rsum[:], rsum_d[:])
            b_pi4 = cpool.tile([128, 1], F32, tag="b_pi4")
            b_3pi4 = cpool.tile([128, 1], F32, tag="b_3pi4")
            nc.gpsimd.memset(b_pi4[:], float(np.pi / 4))
            nc.gpsimd.memset(b_3pi4[:], float(3 * np.pi / 4))

            # persistent sample-major staging
            Tg = ppool.tile([128, 4 * NQ], F16, tag="tg")      # [mu, NQ*i + Q]
            Sg = ppool.tile([128, 4 * NQ], F16, tag="sg")
            Cg = ppool.tile([128, 4 * NQ], F16, tag="cg")
            PSI = ppool.tile([128, 16 * NQ], F16, tag="psi")   # [mu, 16Q + st]
            P01 = [ppool.tile([128, NQ], F16, tag=f"p01_{k}", name=f"P01_{k}") for k in range(4)]
            P23 = [ppool.tile([128, NQ], F16, tag=f"p23_{k}", name=f"P23_{k}") for k in range(4)]
            n_t = max(nm // 2, 1)
            Yo_s = [ppool.tile([128, 512], F16, tag=f"yos{t}", name=f"Yo_s{t}") for t in range(n_t)]
            Yo2 = [ppool.tile([128, 512], F16, tag=f"yo2{t}", name=f"Yo2_{t}") for t in range(n_t)]

            x_r = x[:].rearrange("(m p k) f -> m p (k f)", m=nm, p=128, k=64)
            XT2s = {}

            def front_a(m):
                """load + cast + X transpose for macro m."""
                Xf = xpool.tile([128, 4096], F32, tag="xf", name=f"Xf{m}")
                nc.gpsimd.dma_start(Xf[:], x_r[m])    # plain f32, line rate
                X = xpool.tile([128, 4096], F16, tag="x", name=f"X{m}")
                if m % 2 == 0:
                    nc.vector.tensor_copy(X[:], Xf[:])
                else:
                    nc.scalar.copy(X[:], Xf[:])
                XT2 = xtpool.tile([128, 4096], F16, tag="xt2", name=f"XT2_{m}")
                xbar(XT2[:].rearrange("q (c p) -> q c p", c=32, p=128), X[:])
                XT2s[m] = XT2

            def front_b(m):
                """pre-matmuls + tanh + regroup for macro m."""
                XT2 = XT2s.pop(m)
                T = spool.tile([128, 1024], F16, tag="t", name=f"T{m}")
                for ut in range(2):
                    U = ps_u.tile([128, 512], F32, tag="u", name=f"U{m}_{ut}")
                    for v in range(4):
                        B = 4 * ut + v
                        nc.tensor.matmul(
                            U[32 * v:32 * v + 32, :], wstack[:],
                            XT2[:, 512 * B:512 * B + 512],
                            tile_position=(0, 32 * v),
                        )
                    nc.scalar.activation(T[:, 512 * ut:512 * ut + 512], U[:],
                                         AF.Tanh, bias=biast[:])
                P2s = spool.tile([128, 1024], F16, tag="p2s", name=f"P2s{m}")
                xbar(P2s[:].rearrange("q (c p) -> q c p", c=8, p=128), T[:])
                # P2s[mu, 128*(4ut+cc) + 32v + 4j + i] -> Tg[mu, NQ*i + Q],
                # Q = 64m + 32ut + 8v + 2cc + j
                p2r = P2s[:].rearrange("p (u c v x) -> p u c v x",
                                       u=2, c=4, v=4, x=32)
                tgr = Tg[:].rearrange(
                    "p (i mm uu vv cc jj) -> p cc vv i mm uu jj",
                    i=4, mm=nm, uu=2, vv=4, cc=4, jj=2)
                for ut in range(2):
                    for j in range(2):
                        src = p2r[:, ut, :, :, 4 * j:4 * j + 4]   # (128,4,4,4)
                        dst = tgr[:, :, :, :, m, ut, j]           # (128,4,4,4)
                        nc.gpsimd.tensor_copy(dst, src)

            def backhalf(mp):
                """sins + products + psi transposes + quad-form for the
                macro-pair Q range [128*mp, 128*mp + 128)."""
                q0, q1 = 128 * mp, 128 * mp + 128
                tg_m = Tg[:].rearrange("p (i q) -> p i q", i=4)[:, :, q0:q1]
                sg_m = Sg[:].rearrange("p (i q) -> p i q", i=4)[:, :, q0:q1]
                cg_m = Cg[:].rearrange("p (i q) -> p i q", i=4)[:, :, q0:q1]
                nc.scalar.activation(sg_m, tg_m, AF.Sin,
                                     bias=b_pi4[:], scale=float(np.pi / 4))
                nc.scalar.activation(cg_m, tg_m, AF.Sin,
                                     bias=b_3pi4[:], scale=float(np.pi / 4))
                aw = []
                for w in range(4):
                    aw.append([Cg[:, w * NQ + q0:w * NQ + q1],
                               Sg[:, w * NQ + q0:w * NQ + q1]])
                for i0 in range(2):
                    for i1 in range(2):
                        nc.gpsimd.tensor_tensor(
                            P01[2 * i0 + i1][:, q0:q1], aw[0][i0], aw[1][i1],
                            ALU.mult)
                        nc.gpsimd.tensor_tensor(
                            P23[2 * i0 + i1][:, q0:q1], aw[2][i0], aw[3][i1],
                            ALU.mult)
                psi_m = PSI[:].rearrange("p (q s) -> p q s", s=16)[:, q0:q1, :]
                for st in range(16):
                    nc.vector.tensor_tensor(
                        psi_m[:, :, st], P01[st >> 2][:, q0:q1],
                        P23[st & 3][:, q0:q1], ALU.mult)
                for mm_ in (2 * mp, 2 * mp + 1):
                    PSIT2 = spool.tile([128, 1024], F16, tag="psts",
                                       name=f"PSIT{mm_}")
                    xbar(PSIT2[:].rearrange("q (c p) -> q c p", c=8, p=128),
                         PSI[:, 1024 * mm_:1024 * mm_ + 1024])
                    for wl in range(2):
                        w = 2 * mm_ + wl
                        rhs = PSIT2[:, 512 * wl:512 * wl + 512]
                        MP = ps_my.tile([128, 512], F32, tag="mp",
                                        name=f"MP{w}")
                        nc.tensor.matmul(MP[:], mbd[:], rhs)
                        DP = spool.tile([128, 512], F16, tag="dp",
                                        name=f"DP{w}")
                        nc.vector.tensor_tensor(DP[:], rhs, MP[:], ALU.mult)
                        Yp = ps_my.tile([128, 512], F32, tag="mp",
                                        name=f"Yp{w}")
                        nc.tensor.matmul(Yp[0:32, :], rsum[:], DP[:],
                                         tile_position=(0, 0))
                        t_idx, w_loc = w // 4, w % 4
                        nc.scalar.activation(
                            Yo_s[t_idx][32 * w_loc:32 * w_loc + 32, :],
                            Yp[0:32, :], AF.Copy, bias=float(post_b))

            # software-pipelined emission: X transposes run ahead so the
            # FIFO HWDGE ring never blocks them behind back-half transposes.
            front_a(0)
            front_a(1)
            front_b(0)
            if nm >= 3:
                front_a(2)
            front_b(1)
            backhalf(0)
            if nm == 4:
                front_a(3)
                front_b(2)
                front_b(3)
                backhalf(1)

            # output fix-up transpose (xbar, fp16), repack, contiguous store
            for t in range(n_t):
                xbar(Yo2[t][:].rearrange("q (c p) -> q c p", c=4, p=128),
                     Yo_s[t][:])
                src_r = Yo2[t][:].rearrange(
                    "p (e wh wl q) -> p wh wl e q", e=4, wh=2, wl=2, q=32
                )[:, :, :, :, 0:8]
                Yo3 = spool.tile([128, 128], F32, tag="yo3", name=f"Yo3_{t}")
                dst_p = Yo3[:].rearrange("p (wh wl e q) -> p wh wl e q",
                                         wh=2, wl=2, e=4, q=8)
                nc.vector.tensor_copy(dst_p, src_r)
                dst_r = y[:].rearrange(
                    "(tt wh mu r) o -> tt mu wh (r o)",
                    tt=n_t, wh=2, mu=128, r=64)[t]
                nc.sync.dma_start(
                    dst_r, Yo3[:].rearrange("p (wh r) -> p wh r", wh=2, r=64))

    return nc


# ---------------------------------------------------------------- entry


def kernel(input_features, pre_w, pre_b, q_params, post_w, post_b):
    global LAST_RESULTS
    x_full = np.ascontiguousarray(np.asarray(input_features, np.float32))
    wst, biast, mbd, rsum, id16, id32 = _consts(
        np.asarray(pre_w, np.float32), np.asarray(pre_b, np.float32),
        np.asarray(q_params, np.float32), np.asarray(post_w, np.float32))
    post_b_f = float(np.asarray(post_b).reshape(-1)[0])

    nc = build(nm=NM, post_b=post_b_f)

    shards = x_full.reshape(N_CORES, S, IN_F)
    in_maps = [
        dict(x=np.ascontiguousarray(shards[c]), wstack=wst, biast=biast,
             mbd=mbd, rsum=rsum)
        for c in range(N_CORES)
    ]
    nc.finalize()
    res = run_bass_kernel_spmd(nc, in_maps, list(range(N_CORES)), trace=TRACE)
    LAST_RESULTS = res
    out = np.concatenate([np.asarray(r["y"]).reshape(S, 1) for r in res.results])
    return out.astype(np.float32)


if __name__ == "__main__":
    print("kernel module OK")



# revision 14
# speedup vs baseline: 1.3591x; 1.3591x over previous
"""Trainium2 Bass kernel for nn_DressedQuantumNet (262144 x 64 -> 262144 x 1).

Math reduction (host, params only): the 4-qubit circuit after the per-sample
input RY layer is a FIXED 16x16 linear map U (depends only on q_params).
With the product state psi_w = (cos phi_w, sin phi_w),
phi_w = (pi/4)*(tanh(u_w)+1), u = x @ pre_w.T + pre_b:

    out = psi^T O psi + post_b,     O = sum_w post_w[w] * U^T Z_w U.

Device pipeline per core (32768 samples), fp16 data paths / fp32 psum:
  DMA-cast x->fp16 -> PE transpose -> pre-matmul (K=128: 2 sample-groups x
  64 feats, M=32 zero-padded) -> tanh(+pre_b) on ACT -> PE transpose
  (sample-major regroup) -> sin x2 on ACT -> pair/state products -> PE
  transpose -> block-diag O matmul -> elementwise dot -> reduce-matmul ->
  output transpose -> contiguous DMA out.

Sample bookkeeping: sample s = 8192*m + 64*p + k, k = 32*ut + 8*v + 2*cc + j.
Sample-major coords: s = 8192*(Q//64) + 64*mu + (Q%64) with mu=p,
Q = 64*m + k. Back half: Q = 32*w + 8*eb + qt.
"""
import sys

import numpy as np

for _p in ("/opt/trn_rl_repo",):
    if _p not in sys.path:
        sys.path.insert(0, _p)

import concourse.bass as bass
import concourse.bacc as bacc
import concourse.hw_specs as _hw_specs

_orig_get_act_tables = _hw_specs.get_activation_tables


def _pinned_act_tables(module_arch):
    tabs = _orig_get_act_tables(module_arch)
    if "silu_and_others" in tabs:
        tabs = {k: (v if k == "silu_and_others" else set())
                for k, v in tabs.items()}
    return tabs


bacc.get_activation_tables = _pinned_act_tables
import concourse.mybir as mybir
from concourse import tile
from concourse.bass_utils import run_bass_kernel_spmd

AF = mybir.ActivationFunctionType
ALU = mybir.AluOpType
F32 = mybir.dt.float32
F16 = mybir.dt.float16

N_CORES = 8
BATCH = 262144
S = BATCH // N_CORES          # 32768 samples per core
NM = S // 8192                # 4 macro-tiles per core
N_QUBITS = 4
Q_DEPTH = 6
IN_F = 64

TRACE = False                 # test.py sets True to collect a profile
LAST_RESULTS = None

# ---------------------------------------------------------------- host math


def _ry(theta):
    c, s = np.cos(theta / 2), np.sin(theta / 2)
    return np.array([[c, -s], [s, c]], dtype=np.float64)


def _lift1(gate, wire):
    ops = [np.eye(2)] * N_QUBITS
    ops[wire] = gate
    out = ops[0]
    for o in ops[1:]:
        out = np.kron(out, o)
    return out


def _cnot(ctrl, tgt):
    U = np.zeros((16, 16))
    for i in range(16):
        bits = [(i >> (N_QUBITS - 1 - w)) & 1 for w in range(N_QUBITS)]
        if bits[ctrl] == 1:
            bits[tgt] ^= 1
        j = sum(b << (N_QUBITS - 1 - w) for w, b in enumerate(bits))
        U[j, i] = 1.0
    return U


def quad_form(q_params, post_w):
    """O (16x16 fp64): out = psi^T O psi + post_b."""
    qw = np.asarray(q_params, dtype=np.float64).reshape(Q_DEPTH, N_QUBITS)
    U = np.eye(16)
    for k in range(Q_DEPTH):
        U = _cnot(0, 1) @ U
        U = _cnot(2, 3) @ U
        U = _cnot(1, 2) @ U
        for w in range(N_QUBITS):
            U = _lift1(_ry(qw[k, w]), w) @ U
    Z = np.diag([1.0, -1.0])
    O = np.zeros((16, 16))
    pw = np.asarray(post_w, dtype=np.float64).reshape(-1)
    for w in range(N_QUBITS):
        O += pw[w] * (U.T @ _lift1(Z, w) @ U)
    return O


def _consts(pre_w, pre_b, q_params, post_w):
    # Wstack (128, 32) f16: [64j + f, 4j + i] = pre_w[i, f]; rest zero.
    wstack = np.zeros((128, 32), dtype=np.float32)
    for j in range(2):
        for i in range(4):
            wstack[64 * j:64 * j + 64, 4 * j + i] = pre_w[i, :]
    # bias (128, 1) f32: row r -> pre_b[r % 4]
    biast = np.tile(np.asarray(pre_b, np.float32).reshape(4), 32)[:, None]
    biast = np.ascontiguousarray(biast, dtype=np.float32)
    # Mbd (128, 128) f16: blockdiag 8 x O (O symmetric)
    O = quad_form(q_params, post_w)
    mbd = np.zeros((128, 128), dtype=np.float32)
    for g in range(8):
        mbd[16 * g:16 * g + 16, 16 * g:16 * g + 16] = O
    # Rsum (128, 32) f16: [16q + st, q] = 1 for q in [0,8); rest zero.
    rsum = np.zeros((128, 32), dtype=np.float32)
    for q in range(8):
        rsum[16 * q:16 * q + 16, q] = 1.0
    ident16 = np.eye(128, dtype=np.float16)
    ident32 = np.eye(128, dtype=np.float32)
    return (wstack.astype(np.float16), biast, mbd.astype(np.float16),
            rsum.astype(np.float16), ident16, ident32)


# ---------------------------------------------------------------- program


def build(nm=NM, post_b=0.0):
    nc = bacc.Bacc()
    s_core = nm * 8192
    NQ = nm * 64                       # Q-column count (= s_core / 128)

    x = nc.declare_dram_parameter("x", (s_core, IN_F), F16, isOutput=False)
    y = nc.declare_dram_parameter("y", (s_core, 1), F32, isOutput=True)
    wstack_d = nc.declare_dram_parameter("wstack", (128, 32), F16, isOutput=False)
    bias_d = nc.declare_dram_parameter("biast", (128, 1), F32, isOutput=False)
    mbd_d = nc.declare_dram_parameter("mbd", (128, 128), F16, isOutput=False)
    rsum_d = nc.declare_dram_parameter("rsum", (128, 32), F16, isOutput=False)

    def xbar(out_ap, in_ap):
        nc.sync.dma_start(out_ap, in_ap, transpose=True)

    with tile.TileContext(nc) as tc:
        with (
            tc.tile_pool(name="const", bufs=1) as cpool,
            tc.tile_pool(name="xt2p", bufs=3) as xtpool,
            tc.tile_pool(name="sb16", bufs=3) as spool,
            tc.tile_pool(name="pers", bufs=1) as ppool,
            tc.tile_pool(name="psu", bufs=3, space="PSUM") as ps_u,
            tc.tile_pool(name="psmy", bufs=3, space="PSUM") as ps_my,
        ):
            # constants
            wstack = cpool.tile([128, 32], F16, tag="wstack")
            biast = cpool.tile([128, 1], F32, tag="biast")
            mbd = cpool.tile([128, 128], F16, tag="mbd")
            rsum = cpool.tile([128, 32], F16, tag="rsum")
            nc.sync.dma_start(wstack[:], wstack_d[:])
            nc.sync.dma_start(biast[:], bias_d[:])
            nc.sync.dma_start(mbd[:], mbd_d[:])
            nc.sync.dma_start(rsum[:], rsum_d[:])
            b_pi4 = cpool.tile([128, 1], F32, tag="b_pi4")
            b_3pi4 = cpool.tile([128, 1], F32, tag="b_3pi4")
            nc.gpsimd.memset(b_pi4[:], float(np.pi / 4))
            nc.gpsimd.memset(b_3pi4[:], float(3 * np.pi / 4))

            # persistent sample-major staging
            Tg = ppool.tile([128, 4 * NQ], F16, tag="tg")      # [mu, NQ*i + Q]
            Sg = ppool.tile([128, 4 * NQ], F16, tag="sg")
            Cg = ppool.tile([128, 4 * NQ], F16, tag="cg")
            PSI = ppool.tile([128, 16 * NQ], F16, tag="psi")   # [mu, 16Q + st]
            P01 = [ppool.tile([128, NQ], F16, tag=f"p01_{k}", name=f"P01_{k}") for k in range(4)]
            P23 = [ppool.tile([128, NQ], F16, tag=f"p23_{k}", name=f"P23_{k}") for k in range(4)]
            n_t = max(nm // 2, 1)
            Yo_s = [ppool.tile([128, 512], F16, tag=f"yos{t}", name=f"Yo_s{t}") for t in range(n_t)]
            Yo2 = [ppool.tile([128, 512], F16, tag=f"yo2{t}", name=f"Yo2_{t}") for t in range(n_t)]

            x_r = x[:].rearrange("(m p k) f -> m p (k f)", m=nm, p=128, k=64)
            XT2s = {}

            def front_a(m):
                """direct DRAM->SBUF transposing DMA for macro m (f16)."""
                XT2 = xtpool.tile([128, 4096], F16, tag="xt2", name=f"XT2_{m}")
                nc.sync.dma_start(
                    XT2[:].rearrange("q (c p) -> q c p", c=32, p=128),
                    x_r[m], transpose=True)
                XT2s[m] = XT2

            def front_b(m):
                """pre-matmuls + tanh + regroup for macro m."""
                XT2 = XT2s.pop(m)
                T = spool.tile([128, 1024], F16, tag="t", name=f"T{m}")
                for ut in range(2):
                    U = ps_u.tile([128, 512], F32, tag="u", name=f"U{m}_{ut}")
                    for v in range(4):
                        B = 4 * ut + v
                        nc.tensor.matmul(
                            U[32 * v:32 * v + 32, :], wstack[:],
                            XT2[:, 512 * B:512 * B + 512],
                            tile_position=(0, 32 * v),
                        )
                    nc.scalar.activation(T[:, 512 * ut:512 * ut + 512], U[:],
                                         AF.Tanh, bias=biast[:])
                P2s = spool.tile([128, 1024], F16, tag="p2s", name=f"P2s{m}")
                xbar(P2s[:].rearrange("q (c p) -> q c p", c=8, p=128), T[:])
                # P2s[mu, 128*(4ut+cc) + 32v + 4j + i] -> Tg[mu, NQ*i + Q],
                # Q = 64m + 32ut + 8v + 2cc + j
                p2r = P2s[:].rearrange("p (u c v x) -> p u c v x",
                                       u=2, c=4, v=4, x=32)
                tgr = Tg[:].rearrange(
                    "p (i mm uu vv cc jj) -> p cc vv i mm uu jj",
                    i=4, mm=nm, uu=2, vv=4, cc=4, jj=2)
                for ut in range(2):
                    for j in range(2):
                        src = p2r[:, ut, :, :, 4 * j:4 * j + 4]   # (128,4,4,4)
                        dst = tgr[:, :, :, :, m, ut, j]           # (128,4,4,4)
                        nc.gpsimd.tensor_copy(dst, src)

            def backhalf(mp):
                """sins + products + psi transposes + quad-form for the
                macro-pair Q range [128*mp, 128*mp + 128)."""
                q0, q1 = 128 * mp, 128 * mp + 128
                tg_m = Tg[:].rearrange("p (i q) -> p i q", i=4)[:, :, q0:q1]
                sg_m = Sg[:].rearrange("p (i q) -> p i q", i=4)[:, :, q0:q1]
                cg_m = Cg[:].rearrange("p (i q) -> p i q", i=4)[:, :, q0:q1]
                nc.scalar.activation(sg_m, tg_m, AF.Sin,
                                     bias=b_pi4[:], scale=float(np.pi / 4))
                nc.scalar.activation(cg_m, tg_m, AF.Sin,
                                     bias=b_3pi4[:], scale=float(np.pi / 4))
                aw = []
                for w in range(4):
                    aw.append([Cg[:, w * NQ + q0:w * NQ + q1],
                               Sg[:, w * NQ + q0:w * NQ + q1]])
                for i0 in range(2):
                    for i1 in range(2):
                        nc.gpsimd.tensor_tensor(
                            P01[2 * i0 + i1][:, q0:q1], aw[0][i0], aw[1][i1],
                            ALU.mult)
                        nc.gpsimd.tensor_tensor(
                            P23[2 * i0 + i1][:, q0:q1], aw[2][i0], aw[3][i1],
                            ALU.mult)
                psi_m = PSI[:].rearrange("p (q s) -> p q s", s=16)[:, q0:q1, :]
                for st in range(16):
                    nc.vector.tensor_tensor(
                        psi_m[:, :, st], P01[st >> 2][:, q0:q1],
                        P23[st & 3][:, q0:q1], ALU.mult)
                for mm_ in (2 * mp, 2 * mp + 1):
                    PSIT2 = spool.tile([128, 1024], F16, tag="psts",
                                       name=f"PSIT{mm_}")
                    xbar(PSIT2[:].rearrange("q (c p) -> q c p", c=8, p=128),
                         PSI[:, 1024 * mm_:1024 * mm_ + 1024])
                    for wl in range(2):
                        w = 2 * mm_ + wl
                        rhs = PSIT2[:, 512 * wl:512 * wl + 512]
                        MP = ps_my.tile([128, 512], F32, tag="mp",
                                        name=f"MP{w}")
                        nc.tensor.matmul(MP[:], mbd[:], rhs)
                        DP = spool.tile([128, 512], F16, tag="dp",
                                        name=f"DP{w}")
                        nc.vector.tensor_tensor(DP[:], rhs, MP[:], ALU.mult)
                        Yp = ps_my.tile([128, 512], F32, tag="mp",
                                        name=f"Yp{w}")
                        nc.tensor.matmul(Yp[0:32, :], rsum[:], DP[:],
                                         tile_position=(0, 0))
                        t_idx, w_loc = w // 4, w % 4
                        nc.scalar.activation(
                            Yo_s[t_idx][32 * w_loc:32 * w_loc + 32, :],
                            Yp[0:32, :], AF.Copy, bias=float(post_b))

            # software-pipelined emission: X transposes run ahead so the
            # FIFO HWDGE ring never blocks them behind back-half transposes.
            front_a(0)
            front_a(1)
            front_b(0)
            if nm >= 3:
                front_a(2)
            front_b(1)
            backhalf(0)
            if nm == 4:
                front_a(3)
                front_b(2)
                front_b(3)
                backhalf(1)

            # output fix-up transpose (xbar, fp16), repack, contiguous store
            for t in range(n_t):
                xbar(Yo2[t][:].rearrange("q (c p) -> q c p", c=4, p=128),
                     Yo_s[t][:])
                src_r = Yo2[t][:].rearrange(
                    "p (e wh wl q) -> p wh wl e q", e=4, wh=2, wl=2, q=32
                )[:, :, :, :, 0:8]
                Yo3 = spool.tile([128, 128], F32, tag="yo3", name=f"Yo3_{t}")
                dst_p = Yo3[:].rearrange("p (wh wl e q) -> p wh wl e q",
                                         wh=2, wl=2, e=4, q=8)
                nc.vector.tensor_copy(dst_p, src_r)
                dst_r = y[:].rearrange(
                    "(tt wh mu r) o -> tt mu wh (r o)",
                    tt=n_t, wh=2, mu=128, r=64)[t]
                nc.sync.dma_start(
                    dst_r, Yo3[:].rearrange("p (wh r) -> p wh r", wh=2, r=64))

    return nc


# ---------------------------------------------------------------- entry


def kernel(input_features, pre_w, pre_b, q_params, post_w, post_b):
    global LAST_RESULTS
    x_full = np.ascontiguousarray(
        np.asarray(input_features, np.float32).astype(np.float16))
    wst, biast, mbd, rsum, id16, id32 = _consts(
        np.asarray(pre_w, np.float32), np.asarray(pre_b, np.float32),
        np.asarray(q_params, np.float32), np.asarray(post_w, np.float32))
    post_b_f = float(np.asarray(post_b).reshape(-1)[0])

    nc = build(nm=NM, post_b=post_b_f)

    shards = x_full.reshape(N_CORES, S, IN_F)
    in_maps = [
        dict(x=np.ascontiguousarray(shards[c]), wstack=wst, biast=biast,
             mbd=mbd, rsum=rsum)
        for c in range(N_CORES)
    ]
    nc.finalize()
    res = run_bass_kernel_spmd(nc, in_maps, list(range(N_CORES)), trace=TRACE)
    LAST_RESULTS = res
    out = np.concatenate([np.asarray(r["y"]).reshape(S, 1) for r in res.results])
    return out.astype(np.float32)


if __name__ == "__main__":
    print("kernel module OK")

